# revision 28
# baseline (speedup 1.0000x reference)
"""Multi-head attention (B=8, N=1024, C=1024, H=16) on 8 TRN2 NeuronCores.

Sharding: data-parallel over batch B=8 -> one batch element per core.
Each core computes, for its batch element:
    qkv = x @ qkv_w.T ; q,k,v split ; per-head softmax(q k^T / sqrt(hd)) v

Device-side layout strategy (all matmuls contract over the SBUF partition dim):
  - host passes xT = x[b].T (bf16)     [C, N]   (c on partitions)
  - host passes wT = qkv_w.T (bf16)    [C, 3C]  (c on partitions)
  - qT/kT computed transposed          [d, n]   (head-dim on partitions)
  - v computed in natural layout       [n, dv]  (tokens on partitions), augmented
    with a ones-column so the PV matmul also yields the softmax denominator
  - scores computed transposed S^T=[j,i]; exp fused into the PSUM->SBUF copy
    on the scalar engine (bf16 out); O^T = v_aug.T @ E^T gives [hd+1, i] with
    row 64 the softmax row-sum; reciprocal done at [128,4] layout via a DRAM
    bounce, broadcast back via a stride-0 DRAM read.
  - host transposes the returned outT back to [n, c].

The attention phase is emitted as ONE globally software-pipelined stream:
score-chunks (2 matmuls + 1 exp), PV-chunks lagging a few chunks behind
(so the scalar-engine exp latency never stalls the PE), and the next pair's
qk-projection matmuls sprinkled two-per-chunk. PSUM: one shared 2-bank tag
(3 bufs) for scores+projection, and a 1-bank tag (2 bufs) for PV outputs,
which are released quickly by a staging copy before the normalization chain.

All matmul inputs are bf16 (PSUM accumulation in fp32); measured end-to-end
relative error ~5e-3 vs the fp32 reference (gate is 2e-2).
"""

import sys
from collections import deque

sys.path.insert(0, "/opt/trn_rl_repo")

import ml_dtypes
import numpy as np

import concourse.bacc as bacc
import concourse.mybir as mybir
import concourse.tile as tile
from concourse.bass_utils import run_bass_kernel_spmd

F32 = mybir.dt.float32
BF16 = mybir.dt.bfloat16
EXP = mybir.ActivationFunctionType.Exp

N = 1024  # tokens
C = 1024  # channels
H = 16    # heads
HD = 64   # head dim
NB = 2    # n blocks of 512
CT = 8    # c tiles of 128
SCALE = HD ** -0.5
PV_LAG = 6  # chunks the PV stream lags behind the score stream


def build_nc():
    nc = bacc.Bacc(None, target_bir_lowering=False)
    xT_ext = nc.declare_dram_parameter("xT", [C, N], BF16, isOutput=False)
    # host-packed qk weights: [pair, p, co, 256] (q cols 0:128, k cols 128:256)
    wqk_ext = nc.declare_dram_parameter("wqk", [8, 128, CT, 256], BF16,
                                        isOutput=False)
    wv_ext = nc.declare_dram_parameter("wv", [C, C], BF16, isOutput=False)
    outT_ext = nc.declare_dram_parameter("outT", [C, N], F32, isOutput=True)

    xT3 = xT_ext.rearrange("(co p) n -> p co n", p=128)    # [128, 8, 1024]
    wv3 = wv_ext.rearrange("(co p) d -> p co d", p=128)    # [128, 8, 1024]

    with tile.TileContext(nc) as tc:
        with (
            tc.tile_pool(name="singles", bufs=1) as singles,
            tc.tile_pool(name="psum", bufs=1, space="PSUM") as psum,
            tc.tile_pool(name="drp", bufs=10, space="DRAM") as drp,
            tc.tile_pool(name="wqkpool", bufs=2) as wqkpool,
            tc.tile_pool(name="qkpool", bufs=2) as qkpool,
            tc.tile_pool(name="epool", bufs=9) as epool,
            tc.tile_pool(name="opool", bufs=6) as opool,
        ):
            # ---- prologue loads: pair-0 weights first (split so the first
            # projection matmul can start after only a quarter of the load),
            # then xT per c-tile ----
            wqk_head = wqkpool.tile([128, 2, 256], BF16, tag="wqkh",
                                    name="wqk_head")
            nc.sync.dma_start(out=wqk_head, in_=wqk_ext[0, :, 0:2])
            xT_sb = singles.tile([128, CT, N], BF16)
            nc.sync.dma_start(out=xT_sb[:, 0, :], in_=xT3[:, 0, :])
            wqk_rest = wqkpool.tile([128, 6, 256], BF16, tag="wqkr",
                                    name="wqk_rest")
            nc.sync.dma_start(out=wqk_rest, in_=wqk_ext[0, :, 2:8])
            # even c-tiles on the sync ring, odd on the scalar ring: halves
            # both the per-queue issue serialization (~0.65us each) and the
            # per-ring transfer backlog so the pair-0 projection never waits
            for ct in range(1, CT):
                eng = nc.sync if ct % 2 == 0 else nc.scalar
                eng.dma_start(out=xT_sb[:, ct, :], in_=xT3[:, ct, :])

            def wqk0_slice(ct, off):
                if ct < 2:
                    return wqk_head[:, ct, off:off + 128]
                return wqk_rest[:, ct - 2, off:off + 128]

            # v_aug[p, nt, h, 0:64] = v head h rows; v_aug[p, nt, h, 64] = 1.0
            v_aug = singles.tile([128, CT, H, HD + 1], BF16)
            ones16 = singles.tile([128, H], F32)
            nc.vector.memset(ones16, 1.0)


            def ps_tile(name):
                return psum.tile([128, 1024], F32, tag="ps", bufs=3, name=name)

            def load_wqk(t):
                wqk = wqkpool.tile([128, CT, 256], BF16, tag="wqk", name="wqk")
                nc.sync.dma_start(out=wqk, in_=wqk_ext[t])
                return wqk

            def qk_proj_plain(wslice):
                """Non-pipelined qk projection (used for pair 0 only).
                q and k interleaved per c-tile so both ride the same xT DMA
                wave instead of k waiting for q to finish."""
                qT = qkpool.tile([128, N], BF16, tag="qT", name="qT")
                kT = qkpool.tile([128, N], BF16, tag="kT", name="kT")
                psq = ps_tile("qkpsq")
                psk = ps_tile("qkpsk")
                for ct in range(CT):
                    for nb in range(NB):
                        for ps, off in ((psq, 0), (psk, 128)):
                            nc.tensor.matmul(
                                ps[:, nb * 512:(nb + 1) * 512],
                                wslice(ct, off),
                                xT_sb[:, ct, nb * 512:(nb + 1) * 512],
                                start=(ct == 0),
                                stop=(ct == CT - 1),
                                skip_group_check=True,
                            )
                nc.vector.tensor_copy(qT, psq)
                nc.vector.tensor_copy(kT, psk)
                return qT, kT

            # ---- pair 0 qk projection first (so attention starts early) ----
            qkT = qk_proj_plain(wqk0_slice)

            # ---- v projection: v[n, dv] = sum_c x[n,c] wv[dv,c] ----
            with tc.tile_pool(name="wvpool", bufs=1) as wvpool:
                wv_sb = wvpool.tile([128, CT, C], BF16)
                for ct in range(CT):
                    nc.gpsimd.dma_start(out=wv_sb[:, ct, :], in_=wv3[:, ct, :])
                for nt in range(8):
                    ps = ps_tile("vps")
                    for ct in range(CT):
                        for dvb in range(2):
                            nc.tensor.matmul(
                                ps[:, dvb * 512:(dvb + 1) * 512],
                                xT_sb[:, ct, nt * 128:(nt + 1) * 128],
                                wv_sb[:, ct, dvb * 512:(dvb + 1) * 512],
                                start=(ct == 0),
                                stop=(ct == CT - 1),
                                skip_group_check=True,
                            )
                    for dvb in range(2):
                        h0 = dvb * 8
                        nc.vector.tensor_copy(
                            v_aug[:, nt, h0:h0 + 8, 0:HD],
                            ps[:, dvb * 512:(dvb + 1) * 512].rearrange(
                                "p (h e) -> p h e", h=8
                            ),
                        )
                nc.vector.tensor_copy(
                    v_aug[:, :, :, HD],
                    ones16[:, None, :].to_broadcast([128, CT, H]),
                )

            # ---- global software-pipelined attention stream ----
            pending = deque()  # FIFO of emitters: PV chunks and normalizations
            pending2 = deque()  # second-stage normalize (muls), drained later

            def drain(n_keep):
                while len(pending) > n_keep:
                    pending.popleft()()

            def drain2(n_keep):
                while len(pending2) > n_keep:
                    pending2.popleft()()

            def stage_emit(pv, k, store, tail=False):
                """Stage a PV output to SBUF immediately, freeing its bank."""
                def emit():
                    stage = opool.tile([HD + 1, 512], F32, tag=f"stage{k}",
                                       bufs=4, name="stage")
                    if tail and k == 1:
                        nc.scalar.copy(stage, pv)
                    else:
                        nc.vector.tensor_copy(stage, pv)
                    store[k] = stage
                return emit

            def recip_emit(store, bstore, tail=False):
                """Stage 1 of normalization: approximate fp32 reciprocal of
                each head's denominator row at [1,512] (single custom-DVE
                op, ~0.7us), then partition-broadcast to [64,512] on the
                otherwise-idle gpsimd engine. No DMA hops at all."""
                def emit():
                    rcs = []
                    for k, stage in enumerate([store[0], store[1]]):
                        den = opool.tile([1, 512], F32, tag=f"den{k}",
                                         bufs=2, name="den")
                        nc.vector.tensor_copy(den, stage[HD:HD + 1, :])
                        rc = opool.tile([1, 512], F32, tag=f"rc{k}",
                                        bufs=2, name="rc")
                        nc.vector.reciprocal_approx_fast(rc, den)
                        rcs.append(rc)
                    for k, rc in enumerate(rcs):
                        bcast = opool.tile([HD, 512], F32, tag=f"bcast{k}",
                                           bufs=3, name="bcast")
                        nc.gpsimd.partition_broadcast(bcast, rc)
                        bstore[k] = bcast
                return emit

            def mul_emit(t, ibs, store, bstore, tail=False):
                """Stage 2 of normalization, drained well after stage 1 so
                the broadcast tiles are ready and no engine queue blocks:
                multiply each head's staged PV rows by its reciprocal
                broadcast and ship both heads in one [128,512] DMA."""
                def emit():
                    osb = opool.tile([128, 512], F32, tag="osb", bufs=3,
                                     name="osb")
                    for k in range(2):
                        nc.vector.tensor_mul(
                            osb[k * HD:(k + 1) * HD, :], store[k][0:HD, :],
                            bstore[k],
                        )
                    nc.sync.dma_start(
                        out=outT_ext[2 * t * HD:(2 * t + 2) * HD, ibs], in_=osb
                    )
                return emit

            for t in range(8):
                qT, kT = qkT
                # 2 projection matmuls per S-chunk: q during chunks 0..7,
                # k during chunks 8..15 of this pair's 16 chunks
                if t < 7:
                    wqk_next = load_wqk(t + 1)
                    qT_next = qkpool.tile([128, N], BF16, tag="qT", name="qT")
                    kT_next = qkpool.tile([128, N], BF16, tag="kT", name="kT")
                    proj_state = {}

                def proj_step(ci):
                    """4 projection matmuls for pair t+1, packed into chunks
                    0-3 (q) and 5-8 (k). k sits early enough that its
                    PSUM->SBUF cast completes well before the next pair's
                    first score matmul loads kT as stationary."""
                    if t >= 7:
                        return
                    if ci < 4:
                        half, step4 = 0, ci
                    elif 5 <= ci < 9:
                        half, step4 = 1, ci - 5
                    else:
                        return
                    dst, off = ((qT_next, 0), (kT_next, 128))[half]
                    if step4 == 0:
                        proj_state["ps"] = ps_tile("qkps")
                    ps = proj_state["ps"]
                    for cth in range(2):
                        ct = step4 * 2 + cth
                        for nb in range(NB):
                            nc.tensor.matmul(
                                ps[:, nb * 512:(nb + 1) * 512],
                                wqk_next[:, ct, off:off + 128],
                                xT_sb[:, ct, nb * 512:(nb + 1) * 512],
                                start=(ct == 0),
                                stop=(ct == CT - 1),
                                skip_group_check=True,
                            )
                    if step4 == 3:
                        nc.vector.tensor_copy(dst, ps)

                pvs = {}
                stage_stores = {}
                ci = 0
                for ib in range(NB):
                    ibs = slice(ib * 512, (ib + 1) * 512)
                    for hh in range(2):
                        pvs[(ib, hh)] = psum.tile(
                            [HD + 1, 512], F32, tag="pv", bufs=2, name=f"pv{hh}"
                        )
                    for g in range(4):
                        for hh in range(2):
                            p0 = hh * 64
                            sps = ps_tile(f"sps{hh}")
                            for jh in range(2):
                                jt = 2 * g + jh
                                nc.tensor.matmul(
                                    sps[:, jh * 512:(jh + 1) * 512],
                                    kT[p0:p0 + 64, jt * 128:(jt + 1) * 128],
                                    qT[p0:p0 + 64, ibs],
                                    start=True,
                                    stop=True,
                                )
                            e = epool.tile([128, 2, 512], BF16, tag=f"E{hh}",
                                           name=f"E{hh}")
                            nc.scalar.activation(
                                out=e.rearrange("p j f -> p (j f)"), in_=sps,
                                func=EXP, scale=SCALE,
                            )

                            def pv_emit(e=e, g=g, hh=hh, pv=pvs[(ib, hh)],
                                        h=2 * t + hh):
                                def emit():
                                    for jh in range(2):
                                        jt = 2 * g + jh
                                        nc.tensor.matmul(
                                            pv,
                                            v_aug[:, jt, h, :],
                                            e[:, jh, :],
                                            start=(g == 0 and jh == 0),
                                            stop=(g == 3 and jh == 1),
                                            skip_group_check=True,
                                        )
                                return emit

                            pending.append(pv_emit())
                            if g == 3:
                                store = stage_stores.setdefault(ib, {})
                                pending.append(stage_emit(
                                    pvs[(ib, hh)], hh, store, tail=(t == 7)))
                            if g == 3 and hh == 1:
                                bstore = {}
                                pending.append(recip_emit(
                                    stage_stores[ib], bstore, tail=(t == 7)))
                                pending2.append(mul_emit(
                                    t, ibs, stage_stores[ib], bstore,
                                    tail=(t == 7)))
                            proj_step(ci)
                            ci += 1
                            drain(PV_LAG if t < 7 else (4 if ib == 0 else 2))
                            drain2(1)
                if t < 7:
                    qkT = (qT_next, kT_next)
            drain(0)
            drain2(0)
    nc.compile()
    return nc


_NC_CACHE = {}


def _get_nc():
    if "nc" not in _NC_CACHE:
        _NC_CACHE["nc"] = build_nc()
    return _NC_CACHE["nc"]


def kernel(x: np.ndarray, qkv_w: np.ndarray, _trace: bool = False):
    B = x.shape[0]
    assert x.shape == (B, N, C) and qkv_w.shape == (3 * C, C)
    bf = ml_dtypes.bfloat16
    # pack q,k weights: [pair, p, co, 256]; c = co*128 + p
    wq = qkv_w[0:C].T.reshape(CT, 128, 8, 128).transpose(2, 1, 0, 3)
    wk = qkv_w[C:2 * C].T.reshape(CT, 128, 8, 128).transpose(2, 1, 0, 3)
    wqk = np.ascontiguousarray(
        np.concatenate([wq, wk], axis=3)).astype(bf)
    wv = np.ascontiguousarray(qkv_w[2 * C:3 * C].T).astype(bf)
    in_maps = [
        {"xT": np.ascontiguousarray(x[b].T).astype(bf), "wqk": wqk, "wv": wv}
        for b in range(B)
    ]
    nc = _get_nc()
    res = run_bass_kernel_spmd(
        nc, in_maps, core_ids=list(range(8)), trace=_trace
    )
    out = np.stack([res.results[b]["outT"].T for b in range(B)])
    if _trace:
        return out, res
    return out



# revision 29
# speedup vs baseline: 1.0018x; 1.0018x over previous
"""Multi-head attention (B=8, N=1024, C=1024, H=16) on 8 TRN2 NeuronCores.

Sharding: data-parallel over batch B=8 -> one batch element per core.
Each core computes, for its batch element:
    qkv = x @ qkv_w.T ; q,k,v split ; per-head softmax(q k^T / sqrt(hd)) v

Device-side layout strategy (all matmuls contract over the SBUF partition dim):
  - host passes xT = x[b].T (bf16)     [C, N]   (c on partitions)
  - host passes wT = qkv_w.T (bf16)    [C, 3C]  (c on partitions)
  - qT/kT computed transposed          [d, n]   (head-dim on partitions)
  - v computed in natural layout       [n, dv]  (tokens on partitions), augmented
    with a ones-column so the PV matmul also yields the softmax denominator
  - scores computed transposed S^T=[j,i]; exp fused into the PSUM->SBUF copy
    on the scalar engine (bf16 out); O^T = v_aug.T @ E^T gives [hd+1, i] with
    row 64 the softmax row-sum; reciprocal done at [128,4] layout via a DRAM
    bounce, broadcast back via a stride-0 DRAM read.
  - host transposes the returned outT back to [n, c].

The attention phase is emitted as ONE globally software-pipelined stream:
score-chunks (2 matmuls + 1 exp), PV-chunks lagging a few chunks behind
(so the scalar-engine exp latency never stalls the PE), and the next pair's
qk-projection matmuls sprinkled two-per-chunk. PSUM: one shared 2-bank tag
(3 bufs) for scores+projection, and a 1-bank tag (2 bufs) for PV outputs,
which are released quickly by a staging copy before the normalization chain.

All matmul inputs are bf16 (PSUM accumulation in fp32); measured end-to-end
relative error ~5e-3 vs the fp32 reference (gate is 2e-2).
"""

import sys
from collections import deque

sys.path.insert(0, "/opt/trn_rl_repo")

import ml_dtypes
import numpy as np

import concourse.bacc as bacc
import concourse.mybir as mybir
import concourse.tile as tile
from concourse.bass_utils import run_bass_kernel_spmd

F32 = mybir.dt.float32
BF16 = mybir.dt.bfloat16
EXP = mybir.ActivationFunctionType.Exp

N = 1024  # tokens
C = 1024  # channels
H = 16    # heads
HD = 64   # head dim
NB = 2    # n blocks of 512
CT = 8    # c tiles of 128
SCALE = HD ** -0.5
PV_LAG = 6  # chunks the PV stream lags behind the score stream


def build_nc():
    nc = bacc.Bacc(None, target_bir_lowering=False)
    xT_ext = nc.declare_dram_parameter("xT", [C, N], BF16, isOutput=False)
    # host-packed qk weights: [pair, p, co, 256] (q cols 0:128, k cols 128:256)
    wqk_ext = nc.declare_dram_parameter("wqk", [8, 128, CT, 256], BF16,
                                        isOutput=False)
    wv_ext = nc.declare_dram_parameter("wv", [C, C], BF16, isOutput=False)
    outT_ext = nc.declare_dram_parameter("outT", [C, N], F32, isOutput=True)

    xT3 = xT_ext.rearrange("(co p) n -> p co n", p=128)    # [128, 8, 1024]
    wv3 = wv_ext.rearrange("(co p) d -> p co d", p=128)    # [128, 8, 1024]

    with tile.TileContext(nc) as tc:
        with (
            tc.tile_pool(name="singles", bufs=1) as singles,
            tc.tile_pool(name="psum", bufs=1, space="PSUM") as psum,
            tc.tile_pool(name="drp", bufs=10, space="DRAM") as drp,
            tc.tile_pool(name="wqkpool", bufs=2) as wqkpool,
            tc.tile_pool(name="qkpool", bufs=2) as qkpool,
            tc.tile_pool(name="epool", bufs=9) as epool,
            tc.tile_pool(name="opool", bufs=6) as opool,
        ):
            # ---- prologue loads: pair-0 weights first (split so the first
            # projection matmul can start after only a quarter of the load),
            # then xT per c-tile ----
            # wqk rides the sync ring while xT rides the scalar ring, so
            # the first projection matmul's two inputs transfer in parallel
            # and each ring's issue serialization (~0.65us per dma_start)
            # stays off the other's critical path
            wqk_head = wqkpool.tile([128, 2, 256], BF16, tag="wqkh",
                                    name="wqk_head")
            nc.sync.dma_start(out=wqk_head, in_=wqk_ext[0, :, 0:2])
            xT_sb = singles.tile([128, CT, N], BF16)
            nc.scalar.dma_start(out=xT_sb[:, 0, :], in_=xT3[:, 0, :])
            wqk_rest = wqkpool.tile([128, 6, 256], BF16, tag="wqkr",
                                    name="wqk_rest")
            nc.sync.dma_start(out=wqk_rest, in_=wqk_ext[0, :, 2:8])
            for ct in range(1, CT):
                eng = nc.scalar if ct % 2 == 1 else nc.sync
                eng.dma_start(out=xT_sb[:, ct, :], in_=xT3[:, ct, :])

            def wqk0_slice(ct, off):
                if ct < 2:
                    return wqk_head[:, ct, off:off + 128]
                return wqk_rest[:, ct - 2, off:off + 128]

            # v_aug[p, nt, h, 0:64] = v head h rows; v_aug[p, nt, h, 64] = 1.0
            v_aug = singles.tile([128, CT, H, HD + 1], BF16)
            ones16 = singles.tile([128, H], F32)
            nc.vector.memset(ones16, 1.0)


            def ps_tile(name):
                return psum.tile([128, 1024], F32, tag="ps", bufs=3, name=name)

            def load_wqk(t):
                wqk = wqkpool.tile([128, CT, 256], BF16, tag="wqk", name="wqk")
                nc.sync.dma_start(out=wqk, in_=wqk_ext[t])
                return wqk

            def qk_proj_plain(wslice):
                """Non-pipelined qk projection (used for pair 0 only).
                q and k interleaved per c-tile so both ride the same xT DMA
                wave instead of k waiting for q to finish."""
                qT = qkpool.tile([128, N], BF16, tag="qT", name="qT")
                kT = qkpool.tile([128, N], BF16, tag="kT", name="kT")
                psq = ps_tile("qkpsq")
                psk = ps_tile("qkpsk")
                for ct in range(CT):
                    for nb in range(NB):
                        for ps, off in ((psq, 0), (psk, 128)):
                            nc.tensor.matmul(
                                ps[:, nb * 512:(nb + 1) * 512],
                                wslice(ct, off),
                                xT_sb[:, ct, nb * 512:(nb + 1) * 512],
                                start=(ct == 0),
                                stop=(ct == CT - 1),
                                skip_group_check=True,
                            )
                nc.vector.tensor_copy(qT, psq)
                nc.vector.tensor_copy(kT, psk)
                return qT, kT

            # ---- pair 0 qk projection first (so attention starts early) ----
            qkT = qk_proj_plain(wqk0_slice)

            # ---- v projection: v[n, dv] = sum_c x[n,c] wv[dv,c] ----
            with tc.tile_pool(name="wvpool", bufs=1) as wvpool:
                wv_sb = wvpool.tile([128, CT, C], BF16)
                for ct in range(CT):
                    nc.gpsimd.dma_start(out=wv_sb[:, ct, :], in_=wv3[:, ct, :])
                for nt in range(8):
                    ps = ps_tile("vps")
                    for ct in range(CT):
                        for dvb in range(2):
                            nc.tensor.matmul(
                                ps[:, dvb * 512:(dvb + 1) * 512],
                                xT_sb[:, ct, nt * 128:(nt + 1) * 128],
                                wv_sb[:, ct, dvb * 512:(dvb + 1) * 512],
                                start=(ct == 0),
                                stop=(ct == CT - 1),
                                skip_group_check=True,
                            )
                    for dvb in range(2):
                        h0 = dvb * 8
                        nc.vector.tensor_copy(
                            v_aug[:, nt, h0:h0 + 8, 0:HD],
                            ps[:, dvb * 512:(dvb + 1) * 512].rearrange(
                                "p (h e) -> p h e", h=8
                            ),
                        )
                nc.vector.tensor_copy(
                    v_aug[:, :, :, HD],
                    ones16[:, None, :].to_broadcast([128, CT, H]),
                )

            # ---- global software-pipelined attention stream ----
            pending = deque()  # FIFO of emitters: PV chunks and normalizations
            pending2 = deque()  # second-stage normalize (muls), drained later

            def drain(n_keep):
                while len(pending) > n_keep:
                    pending.popleft()()

            def drain2(n_keep):
                while len(pending2) > n_keep:
                    pending2.popleft()()

            def stage_emit(pv, k, store, tail=False):
                """Stage a PV output to SBUF immediately, freeing its bank."""
                def emit():
                    stage = opool.tile([HD + 1, 512], F32, tag=f"stage{k}",
                                       bufs=4, name="stage")
                    if tail and k == 1:
                        nc.scalar.copy(stage, pv)
                    else:
                        nc.vector.tensor_copy(stage, pv)
                    store[k] = stage
                return emit

            def recip_emit(store, bstore, pvd=None, tail=False):
                """Stage 1 of normalization: approximate fp32 reciprocal of
                each head's denominator row at [1,512] (single custom-DVE
                op, ~0.7us), then partition-broadcast to [64,512] on the
                otherwise-idle gpsimd engine. No DMA hops at all."""
                def emit():
                    rcs = []
                    for k in range(2):
                        src_row = (pvd[k] if tail else store[k])[HD:HD + 1, :]
                        den = opool.tile([1, 512], F32, tag=f"den{k}",
                                         bufs=2, name="den")
                        nc.vector.tensor_copy(den, src_row)
                        rc = opool.tile([1, 512], F32, tag=f"rc{k}",
                                        bufs=2, name="rc")
                        nc.vector.reciprocal_approx_fast(rc, den)
                        rcs.append(rc)
                    for k, rc in enumerate(rcs):
                        bcast = opool.tile([HD, 512], F32, tag=f"bcast{k}",
                                           bufs=3, name="bcast")
                        nc.gpsimd.partition_broadcast(bcast, rc)
                        bstore[k] = bcast
                return emit

            def mul_emit(t, ibs, store, bstore, tail=False):
                """Stage 2 of normalization, drained well after stage 1 so
                the broadcast tiles are ready and no engine queue blocks:
                multiply each head's staged PV rows by its reciprocal
                broadcast and ship both heads in one [128,512] DMA."""
                def emit():
                    osb = opool.tile([128, 512], F32, tag="osb", bufs=3,
                                     name="osb")
                    for k in range(2):
                        nc.vector.tensor_mul(
                            osb[k * HD:(k + 1) * HD, :], store[k][0:HD, :],
                            bstore[k],
                        )
                        if tail:
                            nc.sync.dma_start(
                                out=outT_ext[(2 * t + k) * HD:
                                             (2 * t + k + 1) * HD, ibs],
                                in_=osb[k * HD:(k + 1) * HD, :],
                            )
                    if not tail:
                        nc.sync.dma_start(
                            out=outT_ext[2 * t * HD:(2 * t + 2) * HD, ibs],
                            in_=osb,
                        )
                return emit

            for t in range(8):
                qT, kT = qkT
                # 2 projection matmuls per S-chunk: q during chunks 0..7,
                # k during chunks 8..15 of this pair's 16 chunks
                if t < 7:
                    wqk_next = load_wqk(t + 1)
                    qT_next = qkpool.tile([128, N], BF16, tag="qT", name="qT")
                    kT_next = qkpool.tile([128, N], BF16, tag="kT", name="kT")
                    proj_state = {}

                def proj_step(ci):
                    """4 projection matmuls for pair t+1, packed into chunks
                    0-3 (q) and 5-8 (k). k sits early enough that its
                    PSUM->SBUF cast completes well before the next pair's
                    first score matmul loads kT as stationary."""
                    if t >= 7:
                        return
                    if ci < 4:
                        half, step4 = 0, ci
                    elif 5 <= ci < 9:
                        half, step4 = 1, ci - 5
                    else:
                        return
                    dst, off = ((qT_next, 0), (kT_next, 128))[half]
                    if step4 == 0:
                        proj_state["ps"] = ps_tile("qkps")
                    ps = proj_state["ps"]
                    for cth in range(2):
                        ct = step4 * 2 + cth
                        for nb in range(NB):
                            nc.tensor.matmul(
                                ps[:, nb * 512:(nb + 1) * 512],
                                wqk_next[:, ct, off:off + 128],
                                xT_sb[:, ct, nb * 512:(nb + 1) * 512],
                                start=(ct == 0),
                                stop=(ct == CT - 1),
                                skip_group_check=True,
                            )
                    if step4 == 3:
                        nc.vector.tensor_copy(dst, ps)

                pvs = {}
                stage_stores = {}
                ci = 0
                for ib in range(NB):
                    ibs = slice(ib * 512, (ib + 1) * 512)
                    for hh in range(2):
                        pvs[(ib, hh)] = psum.tile(
                            [HD + 1, 512], F32, tag="pv", bufs=2, name=f"pv{hh}"
                        )
                    for g in range(4):
                        for hh in range(2):
                            p0 = hh * 64
                            sps = ps_tile(f"sps{hh}")
                            for jh in range(2):
                                jt = 2 * g + jh
                                nc.tensor.matmul(
                                    sps[:, jh * 512:(jh + 1) * 512],
                                    kT[p0:p0 + 64, jt * 128:(jt + 1) * 128],
                                    qT[p0:p0 + 64, ibs],
                                    start=True,
                                    stop=True,
                                )
                            e = epool.tile([128, 2, 512], BF16, tag=f"E{hh}",
                                           name=f"E{hh}")
                            nc.scalar.activation(
                                out=e.rearrange("p j f -> p (j f)"), in_=sps,
                                func=EXP, scale=SCALE,
                            )

                            def pv_emit(e=e, g=g, hh=hh, pv=pvs[(ib, hh)],
                                        h=2 * t + hh):
                                def emit():
                                    for jh in range(2):
                                        jt = 2 * g + jh
                                        nc.tensor.matmul(
                                            pv,
                                            v_aug[:, jt, h, :],
                                            e[:, jh, :],
                                            start=(g == 0 and jh == 0),
                                            stop=(g == 3 and jh == 1),
                                            skip_group_check=True,
                                        )
                                return emit

                            pending.append(pv_emit())
                            if g == 3:
                                store = stage_stores.setdefault(ib, {})
                                pending.append(stage_emit(
                                    pvs[(ib, hh)], hh, store, tail=(t == 7)))
                            if g == 3 and hh == 1:
                                bstore = {}
                                pending.append(recip_emit(
                                    stage_stores[ib], bstore,
                                    pvd=(pvs[(ib, 0)], pvs[(ib, 1)]),
                                    tail=(t == 7)))
                                pending2.append(mul_emit(
                                    t, ibs, stage_stores[ib], bstore,
                                    tail=(t == 7)))
                            proj_step(ci)
                            ci += 1
                            drain(PV_LAG if t < 7 else (4 if ib == 0 else 2))
                            drain2(1)
                if t < 7:
                    qkT = (qT_next, kT_next)
            drain(0)
            drain2(0)
    nc.compile()
    return nc


_NC_CACHE = {}


def _get_nc():
    if "nc" not in _NC_CACHE:
        _NC_CACHE["nc"] = build_nc()
    return _NC_CACHE["nc"]


def kernel(x: np.ndarray, qkv_w: np.ndarray, _trace: bool = False):
    B = x.shape[0]
    assert x.shape == (B, N, C) and qkv_w.shape == (3 * C, C)
    bf = ml_dtypes.bfloat16
    # pack q,k weights: [pair, p, co, 256]; c = co*128 + p
    wq = qkv_w[0:C].T.reshape(CT, 128, 8, 128).transpose(2, 1, 0, 3)
    wk = qkv_w[C:2 * C].T.reshape(CT, 128, 8, 128).transpose(2, 1, 0, 3)
    wqk = np.ascontiguousarray(
        np.concatenate([wq, wk], axis=3)).astype(bf)
    wv = np.ascontiguousarray(qkv_w[2 * C:3 * C].T).astype(bf)
    in_maps = [
        {"xT": np.ascontiguousarray(x[b].T).astype(bf), "wqk": wqk, "wv": wv}
        for b in range(B)
    ]
    nc = _get_nc()
    res = run_bass_kernel_spmd(
        nc, in_maps, core_ids=list(range(8)), trace=_trace
    )
    out = np.stack([res.results[b]["outT"].T for b in range(B)])
    if _trace:
        return out, res
    return out



# revision 31
# speedup vs baseline: 1.0038x; 1.0020x over previous
"""Multi-head attention (B=8, N=1024, C=1024, H=16) on 8 TRN2 NeuronCores.

Sharding: data-parallel over batch B=8 -> one batch element per core.
Each core computes, for its batch element:
    qkv = x @ qkv_w.T ; q,k,v split ; per-head softmax(q k^T / sqrt(hd)) v

Device-side layout strategy (all matmuls contract over the SBUF partition dim):
  - host passes xT = x[b].T (bf16)     [C, N]   (c on partitions)
  - host passes wT = qkv_w.T (bf16)    [C, 3C]  (c on partitions)
  - qT/kT computed transposed          [d, n]   (head-dim on partitions)
  - v computed in natural layout       [n, dv]  (tokens on partitions), augmented
    with a ones-column so the PV matmul also yields the softmax denominator
  - scores computed transposed S^T=[j,i]; exp fused into the PSUM->SBUF copy
    on the scalar engine (bf16 out); O^T = v_aug.T @ E^T gives [hd+1, i] with
    row 64 the softmax row-sum; reciprocal done at [128,4] layout via a DRAM
    bounce, broadcast back via a stride-0 DRAM read.
  - host transposes the returned outT back to [n, c].

The attention phase is emitted as ONE globally software-pipelined stream:
score-chunks (2 matmuls + 1 exp), PV-chunks lagging a few chunks behind
(so the scalar-engine exp latency never stalls the PE), and the next pair's
qk-projection matmuls sprinkled two-per-chunk. PSUM: one shared 2-bank tag
(3 bufs) for scores+projection, and a 1-bank tag (2 bufs) for PV outputs,
which are released quickly by a staging copy before the normalization chain.

All matmul inputs are bf16 (PSUM accumulation in fp32); measured end-to-end
relative error ~5e-3 vs the fp32 reference (gate is 2e-2).
"""

import sys
from collections import deque

sys.path.insert(0, "/opt/trn_rl_repo")

import ml_dtypes
import numpy as np

import concourse.bacc as bacc
import concourse.mybir as mybir
import concourse.tile as tile
from concourse.bass_utils import run_bass_kernel_spmd

F32 = mybir.dt.float32
BF16 = mybir.dt.bfloat16
EXP = mybir.ActivationFunctionType.Exp

N = 1024  # tokens
C = 1024  # channels
H = 16    # heads
HD = 64   # head dim
NB = 2    # n blocks of 512
CT = 8    # c tiles of 128
SCALE = HD ** -0.5
PV_LAG = 6  # chunks the PV stream lags behind the score stream


def build_nc():
    nc = bacc.Bacc(None, target_bir_lowering=False)
    xT_ext = nc.declare_dram_parameter("xT", [C, N], BF16, isOutput=False)
    # host-packed qk weights: [pair, p, co, 256] (q cols 0:128, k cols 128:256)
    wqk_ext = nc.declare_dram_parameter("wqk", [8, 128, CT, 256], BF16,
                                        isOutput=False)
    wv_ext = nc.declare_dram_parameter("wv", [C, C], BF16, isOutput=False)
    outT_ext = nc.declare_dram_parameter("outT", [C, N], F32, isOutput=True)

    xT3 = xT_ext.rearrange("(co p) n -> p co n", p=128)    # [128, 8, 1024]
    wv3 = wv_ext.rearrange("(co p) d -> p co d", p=128)    # [128, 8, 1024]

    with tile.TileContext(nc) as tc:
        with (
            tc.tile_pool(name="singles", bufs=1) as singles,
            tc.tile_pool(name="psum", bufs=1, space="PSUM") as psum,
            tc.tile_pool(name="drp", bufs=10, space="DRAM") as drp,
            tc.tile_pool(name="wqkpool", bufs=2) as wqkpool,
            tc.tile_pool(name="qkpool", bufs=2) as qkpool,
            tc.tile_pool(name="epool", bufs=9) as epool,
            tc.tile_pool(name="opool", bufs=6) as opool,
        ):
            # ---- prologue loads: pair-0 weights first (split so the first
            # projection matmul can start after only a quarter of the load),
            # then xT per c-tile ----
            # wqk rides the sync ring while xT rides the scalar ring, so
            # the first projection matmul's two inputs transfer in parallel
            # and each ring's issue serialization (~0.65us per dma_start)
            # stays off the other's critical path
            wqk_head = wqkpool.tile([128, 2, 256], BF16, tag="wqkh",
                                    name="wqk_head")
            nc.sync.dma_start(out=wqk_head, in_=wqk_ext[0, :, 0:2])
            xT_sb = singles.tile([128, CT, N], BF16)
            nc.scalar.dma_start(out=xT_sb[:, 0, :], in_=xT3[:, 0, :])
            wqk_rest = wqkpool.tile([128, 6, 256], BF16, tag="wqkr",
                                    name="wqk_rest")
            nc.sync.dma_start(out=wqk_rest, in_=wqk_ext[0, :, 2:8])
            for ct in range(1, CT):
                eng = nc.scalar if ct % 2 == 1 else nc.sync
                eng.dma_start(out=xT_sb[:, ct, :], in_=xT3[:, ct, :])

            def wqk0_slice(ct, off):
                if ct < 2:
                    return wqk_head[:, ct, off:off + 128]
                return wqk_rest[:, ct - 2, off:off + 128]

            # v_aug[p, nt, h, 0:64] = v head h rows; v_aug[p, nt, h, 64] = 1.0
            v_aug = singles.tile([128, CT, H, HD + 1], BF16)
            ones16 = singles.tile([128, H], F32)
            nc.vector.memset(ones16, 1.0)


            def ps_tile(name):
                return psum.tile([128, 1024], F32, tag="ps", bufs=3, name=name)

            def load_wqk(t):
                wqk = wqkpool.tile([128, CT, 256], BF16, tag="wqk", name="wqk")
                nc.sync.dma_start(out=wqk, in_=wqk_ext[t])
                return wqk

            def qk_proj_plain(wslice):
                """Non-pipelined qk projection (used for pair 0 only).
                q and k interleaved per c-tile so both ride the same xT DMA
                wave instead of k waiting for q to finish."""
                qT = qkpool.tile([128, N], BF16, tag="qT", name="qT")
                kT = qkpool.tile([128, N], BF16, tag="kT", name="kT")
                psq = ps_tile("qkpsq")
                psk = ps_tile("qkpsk")
                for ct in range(CT):
                    for nb in range(NB):
                        for ps, off in ((psq, 0), (psk, 128)):
                            nc.tensor.matmul(
                                ps[:, nb * 512:(nb + 1) * 512],
                                wslice(ct, off),
                                xT_sb[:, ct, nb * 512:(nb + 1) * 512],
                                start=(ct == 0),
                                stop=(ct == CT - 1),
                                skip_group_check=True,
                            )
                nc.vector.tensor_copy(qT, psq)
                nc.vector.tensor_copy(kT, psk)
                return qT, kT

            # ---- pair 0 qk projection first (so attention starts early) ----
            qkT = qk_proj_plain(wqk0_slice)

            # ---- v projection: v[n, dv] = sum_c x[n,c] wv[dv,c] ----
            with tc.tile_pool(name="wvpool", bufs=1) as wvpool:
                wv_sb = wvpool.tile([128, CT, C], BF16)
                for ct in range(CT):
                    nc.gpsimd.dma_start(out=wv_sb[:, ct, :], in_=wv3[:, ct, :])
                for nt in range(8):
                    ps = ps_tile("vps")
                    for ct in range(CT):
                        for dvb in range(2):
                            nc.tensor.matmul(
                                ps[:, dvb * 512:(dvb + 1) * 512],
                                xT_sb[:, ct, nt * 128:(nt + 1) * 128],
                                wv_sb[:, ct, dvb * 512:(dvb + 1) * 512],
                                start=(ct == 0),
                                stop=(ct == CT - 1),
                                skip_group_check=True,
                            )
                    for dvb in range(2):
                        h0 = dvb * 8
                        nc.vector.tensor_copy(
                            v_aug[:, nt, h0:h0 + 8, 0:HD],
                            ps[:, dvb * 512:(dvb + 1) * 512].rearrange(
                                "p (h e) -> p h e", h=8
                            ),
                        )
                nc.vector.tensor_copy(
                    v_aug[:, :, :, HD],
                    ones16[:, None, :].to_broadcast([128, CT, H]),
                )

            # ---- global software-pipelined attention stream ----
            pending = deque()  # FIFO of emitters: PV chunks and normalizations
            pending2 = deque()  # second-stage normalize (muls), drained later

            def drain(n_keep):
                while len(pending) > n_keep:
                    pending.popleft()()

            def drain2(n_keep):
                while len(pending2) > n_keep:
                    pending2.popleft()()

            def stage_emit(pv, k, store, tail=False):
                """Stage a PV output to SBUF immediately, freeing its bank."""
                def emit():
                    stage = opool.tile([HD + 1, 512], F32, tag=f"stage{k}",
                                       bufs=4, name="stage")
                    if tail and k == 1:
                        nc.scalar.copy(stage, pv)
                    else:
                        nc.vector.tensor_copy(stage, pv)
                    store[k] = stage
                return emit

            def recip_emit(store, bstore, pvd=None, tail=False):
                """Stage 1 of normalization: approximate fp32 reciprocal of
                each head's denominator row at [1,512] (single custom-DVE
                op, ~0.7us), then partition-broadcast to [64,512] on the
                otherwise-idle gpsimd engine. No DMA hops at all."""
                def emit():
                    rcs = []
                    for k in range(2):
                        src_row = (pvd[k] if tail else store[k])[HD:HD + 1, :]
                        den = opool.tile([1, 512], F32, tag=f"den{k}",
                                         bufs=2, name="den")
                        nc.vector.tensor_copy(den, src_row)
                        rc = opool.tile([1, 512], F32, tag=f"rc{k}",
                                        bufs=2, name="rc")
                        nc.vector.reciprocal_approx_fast(rc, den)
                        rcs.append(rc)
                    for k, rc in enumerate(rcs):
                        bcast = opool.tile([HD, 512], F32, tag=f"bcast{k}",
                                           bufs=3, name="bcast")
                        nc.gpsimd.partition_broadcast(bcast, rc)
                        bstore[k] = bcast
                return emit

            def mul_emit(t, ibs, store, bstore, tail=False):
                """Stage 2 of normalization, drained well after stage 1 so
                the broadcast tiles are ready and no engine queue blocks:
                multiply each head's staged PV rows by its reciprocal
                broadcast and ship both heads in one [128,512] DMA."""
                def emit():
                    osb = opool.tile([128, 512], F32, tag="osb", bufs=3,
                                     name="osb")
                    for k in range(2):
                        nc.vector.tensor_mul(
                            osb[k * HD:(k + 1) * HD, :], store[k][0:HD, :],
                            bstore[k],
                        )
                        if tail:
                            nc.sync.dma_start(
                                out=outT_ext[(2 * t + k) * HD:
                                             (2 * t + k + 1) * HD, ibs],
                                in_=osb[k * HD:(k + 1) * HD, :],
                            )
                    if not tail:
                        nc.sync.dma_start(
                            out=outT_ext[2 * t * HD:(2 * t + 2) * HD, ibs],
                            in_=osb,
                        )
                return emit

            for t in range(8):
                qT, kT = qkT
                # 2 projection matmuls per S-chunk: q during chunks 0..7,
                # k during chunks 8..15 of this pair's 16 chunks
                if t < 7:
                    wqk_next = load_wqk(t + 1)
                    qT_next = qkpool.tile([128, N], BF16, tag="qT", name="qT")
                    kT_next = qkpool.tile([128, N], BF16, tag="kT", name="kT")
                    proj_state = {}

                def proj_step(ci):
                    """4 projection matmuls for pair t+1, packed into chunks
                    0-3 (q) and 5-8 (k). k sits early enough that its
                    PSUM->SBUF cast completes well before the next pair's
                    first score matmul loads kT as stationary."""
                    if t >= 7:
                        return
                    if ci < 4:
                        half, step4 = 0, ci
                    elif 5 <= ci < 9:
                        half, step4 = 1, ci - 5
                    else:
                        return
                    dst, off = ((qT_next, 0), (kT_next, 128))[half]
                    if step4 == 0:
                        proj_state["ps"] = ps_tile("qkps")
                    ps = proj_state["ps"]
                    for cth in range(2):
                        ct = step4 * 2 + cth
                        for nb in range(NB):
                            nc.tensor.matmul(
                                ps[:, nb * 512:(nb + 1) * 512],
                                wqk_next[:, ct, off:off + 128],
                                xT_sb[:, ct, nb * 512:(nb + 1) * 512],
                                start=(ct == 0),
                                stop=(ct == CT - 1),
                                skip_group_check=True,
                            )
                    if step4 == 3:
                        nc.vector.tensor_copy(dst, ps)

                pvs = {}
                stage_stores = {}
                ci = 0
                for ib in range(NB):
                    ibs = slice(ib * 512, (ib + 1) * 512)
                    for hh in range(2):
                        pvs[(ib, hh)] = psum.tile(
                            [HD + 1, 512], F32, tag="pv", bufs=2, name=f"pv{hh}"
                        )
                    for g in range(4):
                        for hh in range(2):
                            p0 = hh * 64
                            sps = ps_tile(f"sps{hh}")
                            for jh in range(2):
                                jt = 2 * g + jh
                                nc.tensor.matmul(
                                    sps[:, jh * 512:(jh + 1) * 512],
                                    kT[p0:p0 + 64, jt * 128:(jt + 1) * 128],
                                    qT[p0:p0 + 64, ibs],
                                    start=True,
                                    stop=True,
                                )
                            e = epool.tile([128, 2, 512], BF16, tag=f"E{hh}",
                                           name=f"E{hh}")
                            nc.scalar.activation(
                                out=e.rearrange("p j f -> p (j f)"), in_=sps,
                                func=EXP, scale=SCALE,
                            )

                            def pv_emit(e=e, g=g, hh=hh, pv=pvs[(ib, hh)],
                                        h=2 * t + hh):
                                def emit():
                                    for jh in range(2):
                                        jt = 2 * g + jh
                                        nc.tensor.matmul(
                                            pv,
                                            v_aug[:, jt, h, :],
                                            e[:, jh, :],
                                            start=(g == 0 and jh == 0),
                                            stop=(g == 3 and jh == 1),
                                            skip_group_check=True,
                                        )
                                return emit

                            pending.append(pv_emit())
                            tail = (t == 7)
                            if g == 3:
                                store = stage_stores.setdefault(ib, {})
                                if not (tail and hh == 1):
                                    pending.append(stage_emit(
                                        pvs[(ib, hh)], hh, store, tail=tail))
                            if g == 3 and hh == 1:
                                bstore = {}
                                pending.append(recip_emit(
                                    stage_stores[ib], bstore,
                                    pvd=(pvs[(ib, 0)], pvs[(ib, 1)]),
                                    tail=tail))
                                if tail:
                                    # den row reads PV psum directly, so the
                                    # reciprocal chain can run concurrently
                                    # with this scalar-engine staging copy
                                    pending.append(stage_emit(
                                        pvs[(ib, hh)], hh, store, tail=tail))
                                pending2.append(mul_emit(
                                    t, ibs, stage_stores[ib], bstore,
                                    tail=tail))
                            proj_step(ci)
                            ci += 1
                            drain(PV_LAG if t < 7 else (4 if ib == 0 else 2))
                            # flush pair-6/7-ib0 muls before the last ib's
                            # reciprocal chain owns the vector queue
                            drain2(0 if (t == 7 and ib == 1 and 11 <= ci <= 14)
                                   else 1)
                if t < 7:
                    qkT = (qT_next, kT_next)
            drain(0)
            drain2(0)
    nc.compile()
    return nc


_NC_CACHE = {}


def _get_nc():
    if "nc" not in _NC_CACHE:
        _NC_CACHE["nc"] = build_nc()
    return _NC_CACHE["nc"]


def kernel(x: np.ndarray, qkv_w: np.ndarray, _trace: bool = False):
    B = x.shape[0]
    assert x.shape == (B, N, C) and qkv_w.shape == (3 * C, C)
    bf = ml_dtypes.bfloat16
    # pack q,k weights: [pair, p, co, 256]; c = co*128 + p
    wq = qkv_w[0:C].T.reshape(CT, 128, 8, 128).transpose(2, 1, 0, 3)
    wk = qkv_w[C:2 * C].T.reshape(CT, 128, 8, 128).transpose(2, 1, 0, 3)
    wqk = np.ascontiguousarray(
        np.concatenate([wq, wk], axis=3)).astype(bf)
    wv = np.ascontiguousarray(qkv_w[2 * C:3 * C].T).astype(bf)
    in_maps = [
        {"xT": np.ascontiguousarray(x[b].T).astype(bf), "wqk": wqk, "wv": wv}
        for b in range(B)
    ]
    nc = _get_nc()
    res = run_bass_kernel_spmd(
        nc, in_maps, core_ids=list(range(8)), trace=_trace
    )
    out = np.stack([res.results[b]["outT"].T for b in range(B)])
    if _trace:
        return out, res
    return out

